# revision 16
# baseline (speedup 1.0000x reference)
"""AttentionCTSF Trainium2 Bass kernel — self-contained, 8-core SPMD.

Pipeline: 1x1x1 conv (W1) -> depthwise (1,3,3) conv -> channel shuffle ->
linear mix (W_lin) -> qkv -> L2-normalized channel attention over (h w) ->
W_out.  Sharding: H split 8 x 16 rows/core (halo +-1).  The hw-contraction
of the attention is handled by all-reducing second-moment matrices
(C-grams) of the mix INPUT, so q/k are never materialized; W_lin and the
bias are applied analytically post-reduce.

Shapes: x [2,64,10,128,128] f32, W1 [192,64], Wdw [192,1,1,3,3],
W_lin [80,80], b_lin [80], temperature [8,1,1], W_out [64,64].
"""

import sys
import numpy as np

sys.path.insert(0, "/opt/trn_rl_repo")

import ml_dtypes  # noqa: E402

BF16 = ml_dtypes.bfloat16

B, CDIM, T, H, W = 2, 64, 10, 128, 128
C3, C1G, C2G = 192, 8, 24
HEADS, CT = 8, 80
NCORES = 8
HL, HH, W2 = H // NCORES, H // NCORES + 2, W + 2
POS = HL * W          # 2048 per (b, c2)
NCH = 16              # 128-position chunks per (b, c2)
EPS = 1e-12
HWTOT = float(H * W)

# PW channel order p = (c2, c1):  original o = c1*24 + c2
_P2O = np.array([(p % 8) * 24 + (p // 8) for p in range(C3)], np.int64)
TPAIRS = [(0, 2), (2, 4), (4, 6), (6, 8), (8, 10)]


def _host_prep(x, W1, Wdw, W_lin, b_lin, temperature, W_out):
    x = np.asarray(x, np.float32)
    W1 = np.asarray(W1, np.float32)
    wdw = np.asarray(Wdw, np.float32).reshape(C3, 3, 3)
    Wl = np.asarray(W_lin, np.float32)
    bl = np.asarray(b_lin, np.float32)
    temp = np.asarray(temperature, np.float32).reshape(HEADS)
    Wo = np.asarray(W_out, np.float32)

    W1p = W1[_P2O]
    wdwp = wdw[_P2O]

    w1t = np.ascontiguousarray(W1p.T).astype(BF16)          # [64, 192]
    taps = wdwp.reshape(C3, 9).astype(np.float32)            # [192, 9]
    wlt = np.ascontiguousarray(Wl.T).astype(BF16)            # [i, e]
    wl_rm = Wl.astype(np.float32)                            # [e, i]
    blin = bl.reshape(CT, 1).astype(np.float32)
    blin2 = (2.0 * bl).reshape(CT, 1).astype(np.float32)
    b2hw = (HWTOT * bl * bl).reshape(CT, 1).astype(np.float32)
    blhw = (HWTOT * bl).reshape(CT, 1).astype(np.float32)
    beta_bcast = np.tile(bl[None, :], (CT, 1)).astype(np.float32)
    temp_cols = np.tile(temp[None, :], (CT, 1)).astype(np.float32)
    woutt = np.ascontiguousarray(Wo.T).astype(BF16)

    common = dict(w1t=w1t, taps=taps, wlt=wlt, wl_rm=wl_rm, blin=blin,
                  blin2=blin2, b2hw=b2hw, blhw=blhw, beta_bcast=beta_bcast,
                  temp_cols=temp_cols, woutt=woutt)

    ins = []
    for core in range(NCORES):
        h0 = core * HL - 1
        xs = np.zeros((CDIM, B, T, HH, W2), np.float32)
        lo, hi = max(h0, 0), min(h0 + HH, H)
        xs[:, :, :, lo - h0:hi - h0, 1:W + 1] = \
            x[:, :, :, lo:hi, :].transpose(1, 0, 2, 3, 4)
        ins.append({"xs": xs.astype(BF16), **common})
    return ins


def build_kernel(reps=1, use_for_i=False):
    import concourse.bass as bass
    import concourse.mybir as mybir
    from concourse import bacc
    from concourse.tile import TileContext
    from concourse.masks import make_identity

    dt = mybir.dt
    f32, bf16 = dt.float32, dt.bfloat16
    AX = mybir.AxisListType.X
    OP = mybir.AluOpType
    ACTF = mybir.ActivationFunctionType

    nc = bacc.Bacc("TRN2", target_bir_lowering=False, debug=False)

    xs_d = nc.declare_dram_parameter("xs", [CDIM, B, T, HH, W2], bf16, isOutput=False)
    w1t_d = nc.declare_dram_parameter("w1t", [CDIM, C3], bf16, isOutput=False)
    taps_d = nc.declare_dram_parameter("taps", [C3, 9], f32, isOutput=False)
    wlt_d = nc.declare_dram_parameter("wlt", [CT, CT], bf16, isOutput=False)
    wlrm_d = nc.declare_dram_parameter("wl_rm", [CT, CT], f32, isOutput=False)
    blin_d = nc.declare_dram_parameter("blin", [CT, 1], f32, isOutput=False)
    blin2_d = nc.declare_dram_parameter("blin2", [CT, 1], f32, isOutput=False)
    b2hw_d = nc.declare_dram_parameter("b2hw", [CT, 1], f32, isOutput=False)
    blhw_d = nc.declare_dram_parameter("blhw", [CT, 1], f32, isOutput=False)
    beta_d = nc.declare_dram_parameter("beta_bcast", [CT, CT], f32, isOutput=False)
    tcol_d = nc.declare_dram_parameter("temp_cols", [CT, HEADS], f32, isOutput=False)
    woutt_d = nc.declare_dram_parameter("woutt", [CDIM, CDIM], bf16, isOutput=False)
    out_d = nc.declare_dram_parameter("out", [B, CDIM, T, HL, W], f32, isOutput=True)

    # internal DRAM:
    # y2d [ch=192(c2,c1), t, b, h', w]  (so i=(c1,t) is a uniform stride)
    y2d = nc.dram_tensor("y2d", [C3, T, B, HL, W], bf16)
    aod = nc.dram_tensor("aod", [B, HEADS, CT, POS], bf16)
    vdram = nc.dram_tensor("vdram", [CT, B, HEADS, POS], bf16)
    cin = nc.dram_tensor("cin", [CT, B * HEADS * 2 * 162], f32)
    scr = nc.dram_tensor("scr", [B * HEADS, 2, CT], f32)
    cred_d = nc.dram_tensor("cred", [CT, B * HEADS * 2 * 162], f32,
                            addr_space="Shared")

    with TileContext(nc) as tc:
        from contextlib import ExitStack
        with ExitStack() as ctx:
            P = lambda name, bufs, **kw: ctx.enter_context(
                tc.tile_pool(name=name, bufs=bufs, **kw))
            constp = P("const", 1)
            xinp = P("xin", 2)
            yp = P("y", 2)
            y2p = P("y2", 2)
            slabp = P("slab", 3)
            ymixp = P("ymix", 3)
            vp = P("v", 2)
            csbp = P("csb", 1)
            smallp = P("small", 3)
            aop = P("ao", 2)
            aocmp = P("aocm", 2)
            outp = P("outs", 3)
            ps_a = P("ps_a", 2, space="PSUM")
            ps_b = P("ps_b", 2, space="PSUM")

            # ---------------- constants ----------------
            w1t = constp.tile([CDIM, C3], bf16)
            nc.gpsimd.dma_start(w1t[:], w1t_d[:])
            taps0 = constp.tile([128, 9], f32)
            nc.gpsimd.dma_start(taps0[:], taps_d[0:128, :])
            taps1 = constp.tile([64, 9], f32)
            nc.gpsimd.dma_start(taps1[:], taps_d[128:192, :])
            wlt = constp.tile([CT, CT], bf16)
            nc.gpsimd.dma_start(wlt[:], wlt_d[:])
            wlrm = constp.tile([CT, CT], f32)
            nc.gpsimd.dma_start(wlrm[:], wlrm_d[:])
            blin = constp.tile([CT, 1], f32)
            nc.gpsimd.dma_start(blin[:], blin_d[:])
            blin2 = constp.tile([CT, 1], f32)
            nc.gpsimd.dma_start(blin2[:], blin2_d[:])
            b2hw = constp.tile([CT, 1], f32)
            nc.gpsimd.dma_start(b2hw[:], b2hw_d[:])
            blhw = constp.tile([CT, 1], f32)
            nc.gpsimd.dma_start(blhw[:], blhw_d[:])
            beta = constp.tile([CT, CT], f32)
            nc.gpsimd.dma_start(beta[:], beta_d[:])
            tcol = constp.tile([CT, HEADS], f32)
            nc.gpsimd.dma_start(tcol[:], tcol_d[:])
            woutt = constp.tile([CDIM, CDIM], bf16)
            nc.gpsimd.dma_start(woutt[:], woutt_d[:])
            ident = constp.tile([CT, CT], bf16)
            make_identity(nc, ident[:])

            import contextlib

            for _rep in range(1 if (reps > 1 and use_for_i) else reps):
                _rs = contextlib.ExitStack()
                if reps > 1 and use_for_i:
                    _rs.enter_context(tc.For_i(0, reps, 1))
                # ===== stage 1: PW + DW, streamed over (b, tpair) =====
                for b in range(B):
                    for (t0, t1) in TPAIRS:
                        nt = t1 - t0
                        xt = xinp.tile([CDIM, nt, HH, W2], bf16, tag="xt")
                        nc.gpsimd.dma_start(xt[:], xs_d[:, b, t0:t1, :, :])
                        ncols = [128, 64]
                        ytiles = []
                        for oc in range(2):
                            ncol = ncols[oc]
                            yt = yp.tile([ncol, nt, HH, W2], bf16,
                                         tag=f"y{oc}")
                            flat = nt * HH * W2          # 4680
                            xf = xt[:].rearrange("c t h w -> c (t h w)")
                            yf = yt[:].rearrange("c t h w -> c (t h w)")
                            csz = 468
                            for ch in range(flat // csz):
                                ps = ps_a.tile([ncol, csz], f32, tag="pw")
                                nc.tensor.matmul(
                                    ps[:],
                                    w1t[:, 128 * oc:128 * oc + ncol],
                                    xf[:, ch * csz:(ch + 1) * csz],
                                    start=True, stop=True)
                                if ch % 2:
                                    nc.scalar.copy(
                                        yf[:, ch * csz:(ch + 1) * csz], ps[:])
                                else:
                                    nc.vector.tensor_copy(
                                        yf[:, ch * csz:(ch + 1) * csz], ps[:])
                            ytiles.append(yt)

                        # DW on DVE: 9 taps, shifted slices (w offsets 0..2)
                        for oc in range(2):
                            ncol = ncols[oc]
                            yt = ytiles[oc]
                            tp = taps0 if oc == 0 else taps1
                            y2t = y2p.tile([ncol, nt, HL, W], bf16,
                                           tag=f"y2{oc}")
                            for tt in range(nt):
                                for k in range(9):
                                    dh, dw = divmod(k, 3)
                                    src = yt[:, tt, dh:dh + HL, dw:dw + W]
                                    if k == 0:
                                        nc.vector.tensor_scalar(
                                            y2t[:, tt], src, tp[:, 0:1],
                                            None, OP.mult)
                                    else:
                                        nc.vector.scalar_tensor_tensor(
                                            y2t[:, tt], src, tp[:, k:k + 1],
                                            y2t[:, tt], OP.mult, OP.add)
                            # spill to DRAM (ch-major, i-uniform layout)
                            dst = y2d[128 * oc:128 * oc + ncol, t0:t1, b, :, :]
                            nc.gpsimd.dma_start(
                                dst, y2t[:])

                # ===== stage 2: C-grams (q/k) + v-MIX =====
                for b in range(B):
                    for hd in range(HEADS):
                        slab = slabp.tile([128, 2, NCH, 81], bf16, tag="slab")
                        nc.vector.memset(slab[:, :, :, 80:81], 1.0)
                        for qk in range(2):
                            c2 = hd + 8 * qk
                            # dram [ch=c2*8+c1, t, b, h', w] ->
                            #   [w-part, (h'chunk, i=(c1,t))]
                            src = y2d[8 * c2:8 * c2 + 8, :, b, :, :]
                            src = src.rearrange("c1 t h w -> w h (c1 t)")
                            for chk in range(NCH):
                                nc.gpsimd.dma_start(
                                    slab[:, qk, chk, 0:80], src[:, chk, :])
                        ps = ps_b.tile([CT, 2, 162], f32, tag="sps")
                        for qk in range(2):
                            for chk in range(NCH):
                                nc.tensor.matmul(
                                    ps[:, qk],
                                    slab[:, qk, chk, 0:80],
                                    slab[:, :, chk, :],
                                    start=(chk == 0), stop=(chk == NCH - 1))
                        cpc = smallp.tile([CT, 2, 162], f32, tag="cpiece")
                        nc.vector.tensor_copy(cpc[:], ps[:])
                        col = (b * HEADS + hd) * 324
                        nc.gpsimd.dma_start(
                            cin[:, col:col + 324],
                            cpc[:].rearrange("c a i -> c (a i)"))

                for b in range(B):
                    for c2v in range(HEADS):
                        ym = ymixp.tile([CT, POS], bf16, tag="ymix")
                        src = y2d[8 * (16 + c2v):8 * (16 + c2v) + 8, :, b, :, :]
                        nc.gpsimd.dma_start(
                            ym[:], src.rearrange("c1 t h w -> (c1 t) (h w)"))
                        vt = vp.tile([CT, POS], bf16, tag="vt")
                        for chk in range(POS // 512):
                            ps = ps_b.tile([CT, 512], f32, tag="mm512")
                            nc.tensor.matmul(
                                ps[:], wlt[:],
                                ym[:, 512 * chk:512 * (chk + 1)],
                                start=True, stop=True)
                            nc.scalar.activation(
                                vt[:, 512 * chk:512 * (chk + 1)],
                                ps[:], ACTF.Identity, bias=blin[:, 0:1],
                                scale=1.0)
                        nc.gpsimd.dma_start(vdram[:, b, c2v, :], vt[:])

                # ===== stage 3: all-reduce C =====
                nc.gpsimd.collective_compute(
                    "AllReduce", OP.add,
                    replica_groups=[list(range(NCORES))],
                    ins=[cin[:]], outs=[cred_d[:]])
                credf = csbp.tile([CT, B, HEADS, 2, 162], f32, tag="credsb")
                nc.gpsimd.dma_start(
                    credf[:].rearrange("c b h a i -> c (b h a i)"), cred_d[:])
                cred = csbp.tile([CT, B, HEADS, 2, 162], bf16, tag="credbf")
                nc.vector.tensor_copy(cred[:], credf[:])

                # ===== stage 4: per-(b,hd) attention math + attn@v =====
                for b in range(B):
                    for hd in range(HEADS):
                        # M1 = Wl @ [C | s] for q-pass and k-pass
                        psm = ps_b.tile([CT, 2, 162], f32, tag="sps")
                        for qk in range(2):
                            nc.tensor.matmul(psm[:, qk], wlt[:],
                                             cred[:, b, hd, qk],
                                             start=True, stop=True)
                        m1 = smallp.tile([CT, 2, 162], f32, tag="m1")
                        nc.vector.tensor_copy(m1[:], psm[:])
                        # m1[:,0,:] = [WlCqq | Wl s_q | WlCqk | Wl s_q]
                        # m1[:,1,:] = [WlCqk^T| Wl s_k | WlCkk | Wl s_k]

                        # squared norms: diag(Wl C Wl^T) + 2 beta (Wl s) + hw b^2
                        nrm = smallp.tile([CT, 2], f32, tag="nrm")
                        tmp = smallp.tile([CT, CT], f32, tag="dtmp")
                        for qk in range(2):
                            blk = m1[:, qk, 81 * qk:81 * qk + 80]
                            nc.vector.tensor_tensor(
                                tmp[:], blk, wlrm[:], OP.mult)
                            nc.vector.tensor_reduce(
                                nrm[:, qk:qk + 1], tmp[:], op=OP.add, axis=AX)
                            wls = m1[:, qk, 80 + 81 * qk:81 + 81 * qk]
                            nc.vector.scalar_tensor_tensor(
                                nrm[:, qk:qk + 1], wls, blin2[:, 0:1],
                                nrm[:, qk:qk + 1], OP.mult, OP.add)
                        nc.vector.scalar_tensor_tensor(
                            nrm[:], b2hw[:, 0:1].broadcast_to([CT, 2]), 1.0,
                            nrm[:], OP.mult, OP.add)

                        rr = smallp.tile([CT, 2], f32, tag="rr")
                        nc.scalar.activation(rr[:], nrm[:], ACTF.Sqrt)
                        nc.vector.tensor_scalar_max(rr[:], rr[:], EPS)
                        nc.vector.reciprocal(rr[:], rr[:])
                        rq = smallp.tile([CT, 1], f32, tag="rq")
                        nc.vector.tensor_tensor(
                            rq[:], rr[:, 0:1], tcol[:, hd:hd + 1], OP.mult)

                        # G = Wl Cqk Wl^T + (Wl s_q) b^T + b (Wl s_k)^T + hw b b^T
                        m1qk = smallp.tile([CT, CT], bf16, tag="m1qk")
                        nc.vector.tensor_copy(m1qk[:], m1[:, 0, 81:161])
                        pst = ps_a.tile([CT, CT], bf16, tag="trps")
                        nc.tensor.transpose(pst[:], m1qk[:], ident[:])
                        m1t = smallp.tile([CT, CT], bf16, tag="m1t")
                        nc.vector.tensor_copy(m1t[:], pst[:])
                        psg = ps_b.tile([CT, CT], f32, tag="sps")
                        nc.tensor.matmul(psg[:], m1t[:], wlt[:],
                                         start=True, stop=True)
                        g = smallp.tile([CT, CT], f32, tag="g")
                        nc.vector.tensor_copy(g[:], psg[:])
                        nc.vector.scalar_tensor_tensor(
                            g[:], beta[:], m1[:, 0, 80:81], g[:],
                            OP.mult, OP.add)
                        # b (Wl s_k)^T : broadcast Wl s_k along free dim
                        # (via DRAM scratch; partition->free needs flat side)
                        import concourse.bass as _b
                        slot = b * HEADS + hd
                        nc.gpsimd.dma_start(scr[slot, 0, :], m1[:, 1, 161:162])
                        wlsk = smallp.tile([CT, CT], f32, tag="wlsk")
                        sap = scr[slot, 0, :]
                        nc.gpsimd.dma_start(
                            wlsk[:],
                            _b.AP(tensor=sap.tensor, offset=sap.offset,
                                  ap=[[0, CT], [1, CT]]))
                        nc.vector.scalar_tensor_tensor(
                            g[:], wlsk[:], blin[:, 0:1], g[:],
                            OP.mult, OP.add)
                        nc.vector.scalar_tensor_tensor(
                            g[:], beta[:], blhw[:, 0:1], g[:],
                            OP.mult, OP.add)

                        # logits = G * rq[part] * rk[free]; softmax over free
                        rkb = smallp.tile([CT, CT], f32, tag="rkb")
                        nc.gpsimd.dma_start(scr[slot, 1, :], rr[:, 1:2])
                        sap2 = scr[slot, 1, :]
                        nc.gpsimd.dma_start(
                            rkb[:],
                            _b.AP(tensor=sap2.tensor, offset=sap2.offset,
                                  ap=[[0, CT], [1, CT]]))
                        lg = smallp.tile([CT, CT], f32, tag="lg")
                        nc.vector.scalar_tensor_tensor(
                            lg[:], g[:], rq[:, 0:1], rkb[:], OP.mult, OP.mult)
                        mx = smallp.tile([CT, 1], f32, tag="mx")
                        nc.vector.tensor_reduce(mx[:], lg[:], op=OP.max,
                                                axis=AX, negate=True)
                        ex = smallp.tile([CT, CT], f32, tag="ex")
                        ssum = smallp.tile([CT, 1], f32, tag="ssum")
                        nc.scalar.activation(ex[:], lg[:], ACTF.Exp,
                                             bias=mx[:, 0:1], scale=1.0,
                                             accum_out=ssum[:, 0:1])
                        nc.vector.reciprocal(ssum[:], ssum[:])
                        at = smallp.tile([CT, CT], bf16, tag="at")
                        nc.vector.tensor_scalar(at[:], ex[:], ssum[:, 0:1],
                                                None, OP.mult)
                        # transpose attn for attn@v
                        psT = ps_a.tile([CT, CT], bf16, tag="trps")
                        nc.tensor.transpose(psT[:], at[:], ident[:])
                        atT = smallp.tile([CT, CT], bf16, tag="atTs")
                        nc.vector.tensor_copy(atT[:], psT[:])

                        ao = aop.tile([CT, POS], bf16, tag="ao")
                        vt = vp.tile([CT, POS], bf16, tag="vt")
                        nc.gpsimd.dma_start(vt[:], vdram[:, b, hd, :])
                        for chk in range(POS // 512):
                            ps = ps_b.tile([CT, 512], f32, tag="mm512")
                            nc.tensor.matmul(
                                ps[:], atT[:],
                                vt[:, 512 * chk:512 * (chk + 1)],
                                start=True, stop=True)
                            if chk % 2:
                                nc.scalar.copy(
                                    ao[:, 512 * chk:512 * (chk + 1)], ps[:])
                            else:
                                nc.vector.tensor_copy(
                                    ao[:, 512 * chk:512 * (chk + 1)], ps[:])
                        nc.gpsimd.dma_start(aod[b, hd], ao[:])

                # ===== stage 5: W_out =====
                for b in range(B):
                    src_b = aod[b].rearrange("hd (cc t) p -> (hd cc) t p",
                                             cc=8)
                    for t_i in range(T):
                        aocm = aocmp.tile([CDIM, POS], bf16, tag="aocm")
                        nc.gpsimd.dma_start(aocm[:], src_b[:, t_i, :])
                        for chk in range(POS // 512):
                            ps = ps_b.tile([CDIM, 512], f32, tag="mm512")
                            nc.tensor.matmul(
                                ps[:], woutt[:],
                                aocm[:, 512 * chk:512 * (chk + 1)],
                                start=True, stop=True)
                            ot = outp.tile([CDIM, 512], f32, tag="ot")
                            if chk % 2:
                                nc.scalar.copy(ot[:], ps[:])
                            else:
                                nc.vector.tensor_copy(ot[:], ps[:])
                            h_i = 512 * chk // W
                            nc.gpsimd.dma_start(
                                out_d[b, :, t_i, h_i:h_i + 4, :],
                                ot[:].rearrange("c (h w) -> c h w", w=W))
                _rs.close()

    nc.compile()
    return nc


_CACHED = {}


def _get_runner(reps=1):
    if reps in _CACHED:
        return _CACHED[reps]
    nc = build_kernel(reps)
    _CACHED[reps] = nc
    return nc


def kernel(**inputs) -> np.ndarray:
    from concourse.bass_utils import run_bass_kernel_spmd
    nc = _get_runner(1)
    in_maps = _host_prep(
        inputs["x"], inputs["W1"], inputs["Wdw"], inputs["W_lin"],
        inputs["b_lin"], inputs["temperature"], inputs["W_out"])
    res = run_bass_kernel_spmd(nc, in_maps, list(range(NCORES)))
    shards = [res.results[c]["out"] for c in range(NCORES)]
    return np.ascontiguousarray(
        np.concatenate(shards, axis=3).astype(np.float32))


# revision 17
# speedup vs baseline: 7.0718x; 7.0718x over previous
"""AttentionCTSF Trainium2 Bass kernel — self-contained, 8-core SPMD.

Pipeline: 1x1x1 conv (W1) -> depthwise (1,3,3) conv -> channel shuffle ->
linear mix (W_lin) -> qkv -> L2-normalized channel attention over (h w) ->
W_out.  Sharding: H split 8 x 16 rows/core (halo +-1).  The hw-contraction
of the attention is handled by all-reducing second-moment matrices
(C-grams) of the mix INPUT, so q/k are never materialized; W_lin and the
bias are applied analytically post-reduce.

Shapes: x [2,64,10,128,128] f32, W1 [192,64], Wdw [192,1,1,3,3],
W_lin [80,80], b_lin [80], temperature [8,1,1], W_out [64,64].
"""

import sys
import numpy as np

sys.path.insert(0, "/opt/trn_rl_repo")

import ml_dtypes  # noqa: E402

BF16 = ml_dtypes.bfloat16

B, CDIM, T, H, W = 2, 64, 10, 128, 128
C3, C1G, C2G = 192, 8, 24
HEADS, CT = 8, 80
NCORES = 8
HL, HH, W2 = H // NCORES, H // NCORES + 2, W + 2
POS = HL * W          # 2048 per (b, c2)
NCH = 16              # 128-position chunks per (b, c2)
EPS = 1e-12
HWTOT = float(H * W)

# PW channel order p = (c2, c1):  original o = c1*24 + c2
_P2O = np.array([(p % 8) * 24 + (p // 8) for p in range(C3)], np.int64)
TPAIRS = [(0, 2), (2, 4), (4, 6), (6, 8), (8, 10)]


def _host_prep(x, W1, Wdw, W_lin, b_lin, temperature, W_out):
    x = np.asarray(x, np.float32)
    W1 = np.asarray(W1, np.float32)
    wdw = np.asarray(Wdw, np.float32).reshape(C3, 3, 3)
    Wl = np.asarray(W_lin, np.float32)
    bl = np.asarray(b_lin, np.float32)
    temp = np.asarray(temperature, np.float32).reshape(HEADS)
    Wo = np.asarray(W_out, np.float32)

    W1p = W1[_P2O]
    wdwp = wdw[_P2O]

    w1t = np.ascontiguousarray(W1p.T).astype(BF16)          # [64, 192]
    taps = wdwp.reshape(C3, 9).astype(np.float32)            # [192, 9]
    wlt = np.ascontiguousarray(Wl.T).astype(BF16)            # [i, e]
    wl_rm = Wl.astype(np.float32)                            # [e, i]
    blin = bl.reshape(CT, 1).astype(np.float32)
    blin2 = (2.0 * bl).reshape(CT, 1).astype(np.float32)
    b2hw = (HWTOT * bl * bl).reshape(CT, 1).astype(np.float32)
    blhw = (HWTOT * bl).reshape(CT, 1).astype(np.float32)
    beta_bcast = np.tile(bl[None, :], (CT, 1)).astype(np.float32)
    temp_cols = np.tile(temp[None, :], (CT, 1)).astype(np.float32)
    woutt = np.ascontiguousarray(Wo.T).astype(BF16)

    common = dict(w1t=w1t, taps=taps, wlt=wlt, wl_rm=wl_rm, blin=blin,
                  blin2=blin2, b2hw=b2hw, blhw=blhw, beta_bcast=beta_bcast,
                  temp_cols=temp_cols, woutt=woutt)

    ins = []
    for core in range(NCORES):
        h0 = core * HL - 1
        xs = np.zeros((CDIM, B, T, HH, W2), np.float32)
        lo, hi = max(h0, 0), min(h0 + HH, H)
        xs[:, :, :, lo - h0:hi - h0, 1:W + 1] = \
            x[:, :, :, lo:hi, :].transpose(1, 0, 2, 3, 4)
        ins.append({"xs": xs.astype(BF16), **common})
    return ins


def build_kernel(reps=1, use_for_i=False):
    import concourse.bass as bass
    import concourse.mybir as mybir
    from concourse import bacc
    from concourse.tile import TileContext
    from concourse.masks import make_identity

    dt = mybir.dt
    f32, bf16 = dt.float32, dt.bfloat16
    AX = mybir.AxisListType.X
    OP = mybir.AluOpType
    ACTF = mybir.ActivationFunctionType

    nc = bacc.Bacc("TRN2", target_bir_lowering=False, debug=False)

    xs_d = nc.declare_dram_parameter("xs", [CDIM, B, T, HH, W2], bf16, isOutput=False)
    w1t_d = nc.declare_dram_parameter("w1t", [CDIM, C3], bf16, isOutput=False)
    taps_d = nc.declare_dram_parameter("taps", [C3, 9], f32, isOutput=False)
    wlt_d = nc.declare_dram_parameter("wlt", [CT, CT], bf16, isOutput=False)
    wlrm_d = nc.declare_dram_parameter("wl_rm", [CT, CT], f32, isOutput=False)
    blin_d = nc.declare_dram_parameter("blin", [CT, 1], f32, isOutput=False)
    blin2_d = nc.declare_dram_parameter("blin2", [CT, 1], f32, isOutput=False)
    b2hw_d = nc.declare_dram_parameter("b2hw", [CT, 1], f32, isOutput=False)
    blhw_d = nc.declare_dram_parameter("blhw", [CT, 1], f32, isOutput=False)
    beta_d = nc.declare_dram_parameter("beta_bcast", [CT, CT], f32, isOutput=False)
    tcol_d = nc.declare_dram_parameter("temp_cols", [CT, HEADS], f32, isOutput=False)
    woutt_d = nc.declare_dram_parameter("woutt", [CDIM, CDIM], bf16, isOutput=False)
    out_d = nc.declare_dram_parameter("out", [B, CDIM, T, HL, W], f32, isOutput=True)

    # internal DRAM:
    # y2d [ch=192(c2,c1), t, b, h', w]  (so i=(c1,t) is a uniform stride)
    y2d = nc.dram_tensor("y2d", [C3, T, B, HL, W], bf16)
    aod = nc.dram_tensor("aod", [B, HEADS, CT, POS], bf16)
    vdram = nc.dram_tensor("vdram", [CT, B, HEADS, POS], bf16)
    cin = nc.dram_tensor("cin", [CT, B * HEADS * 2 * 162], f32)
    scr = nc.dram_tensor("scr", [B * HEADS, 2, CT], f32)
    cred_d = nc.dram_tensor("cred", [CT, B * HEADS * 2 * 162], f32,
                            addr_space="Shared")

    with TileContext(nc) as tc:
        from contextlib import ExitStack
        with ExitStack() as ctx:
            P = lambda name, bufs, **kw: ctx.enter_context(
                tc.tile_pool(name=name, bufs=bufs, **kw))
            constp = P("const", 1)
            xinp = P("xin", 2)
            yp = P("y", 2)
            y2p = P("y2", 2)
            slabp = P("slab", 3)
            ymixp = P("ymix", 3)
            vp = P("v", 2)
            csbp = P("csb", 1)
            smallp = P("small", 3)
            aop = P("ao", 2)
            aocmp = P("aocm", 2)
            outp = P("outs", 3)
            ps_a = P("ps_a", 2, space="PSUM")
            ps_b = P("ps_b", 2, space="PSUM")

            # ---------------- constants ----------------
            w1t = constp.tile([CDIM, C3], bf16)
            nc.gpsimd.dma_start(w1t[:], w1t_d[:])
            taps0 = constp.tile([128, 9], f32)
            nc.gpsimd.dma_start(taps0[:], taps_d[0:128, :])
            taps1 = constp.tile([64, 9], f32)
            nc.gpsimd.dma_start(taps1[:], taps_d[128:192, :])
            wlt = constp.tile([CT, CT], bf16)
            nc.gpsimd.dma_start(wlt[:], wlt_d[:])
            wlrm = constp.tile([CT, CT], f32)
            nc.gpsimd.dma_start(wlrm[:], wlrm_d[:])
            blin = constp.tile([CT, 1], f32)
            nc.gpsimd.dma_start(blin[:], blin_d[:])
            blin2 = constp.tile([CT, 1], f32)
            nc.gpsimd.dma_start(blin2[:], blin2_d[:])
            b2hw = constp.tile([CT, 1], f32)
            nc.gpsimd.dma_start(b2hw[:], b2hw_d[:])
            blhw = constp.tile([CT, 1], f32)
            nc.gpsimd.dma_start(blhw[:], blhw_d[:])
            beta = constp.tile([CT, CT], f32)
            nc.gpsimd.dma_start(beta[:], beta_d[:])
            tcol = constp.tile([CT, HEADS], f32)
            nc.gpsimd.dma_start(tcol[:], tcol_d[:])
            woutt = constp.tile([CDIM, CDIM], bf16)
            nc.gpsimd.dma_start(woutt[:], woutt_d[:])
            ident = constp.tile([CT, CT], bf16)
            make_identity(nc, ident[:])

            import contextlib

            for _rep in range(1 if (reps > 1 and use_for_i) else reps):
                _rs = contextlib.ExitStack()
                if reps > 1 and use_for_i:
                    _rs.enter_context(tc.For_i(0, reps, 1))
                # ===== stage 1: PW + DW, streamed over (b, tpair) =====
                for b in range(B):
                    for (t0, t1) in TPAIRS:
                        nt = t1 - t0
                        xt = xinp.tile([CDIM, nt, HH, W2], bf16, tag="xt")
                        nc.gpsimd.dma_start(xt[:], xs_d[:, b, t0:t1, :, :])
                        ncols = [128, 64]
                        ytiles = []
                        for oc in range(2):
                            ncol = ncols[oc]
                            yt = yp.tile([ncol, nt, HH, W2], bf16,
                                         tag=f"y{oc}")
                            flat = nt * HH * W2          # 4680
                            xf = xt[:].rearrange("c t h w -> c (t h w)")
                            yf = yt[:].rearrange("c t h w -> c (t h w)")
                            csz = 468
                            for ch in range(flat // csz):
                                ps = ps_a.tile([ncol, csz], f32, tag="pw")
                                nc.tensor.matmul(
                                    ps[:],
                                    w1t[:, 128 * oc:128 * oc + ncol],
                                    xf[:, ch * csz:(ch + 1) * csz],
                                    start=True, stop=True)
                                if ch % 2:
                                    nc.scalar.copy(
                                        yf[:, ch * csz:(ch + 1) * csz], ps[:])
                                else:
                                    nc.vector.tensor_copy(
                                        yf[:, ch * csz:(ch + 1) * csz], ps[:])
                            ytiles.append(yt)

                        # DW on DVE: 9 taps, shifted slices (w offsets 0..2)
                        for oc in range(2):
                            ncol = ncols[oc]
                            yt = ytiles[oc]
                            tp = taps0 if oc == 0 else taps1
                            y2t = y2p.tile([ncol, nt, HL, W], bf16,
                                           tag=f"y2{oc}")
                            for tt in range(nt):
                                for k in range(9):
                                    dh, dw = divmod(k, 3)
                                    src = yt[:, tt, dh:dh + HL, dw:dw + W]
                                    if k == 0:
                                        nc.vector.tensor_scalar(
                                            y2t[:, tt], src, tp[:, 0:1],
                                            None, OP.mult)
                                    else:
                                        nc.vector.scalar_tensor_tensor(
                                            y2t[:, tt], src, tp[:, k:k + 1],
                                            y2t[:, tt], OP.mult, OP.add)
                            # spill to DRAM (ch-major, i-uniform layout)
                            dst = y2d[128 * oc:128 * oc + ncol, t0:t1, b, :, :]
                            nc.gpsimd.dma_start(
                                dst, y2t[:])

                # ===== stage 2: C-grams (q/k) + v-MIX =====
                for b in range(B):
                    for hd in range(HEADS):
                        slab = slabp.tile([128, 2, NCH, 81], bf16, tag="slab")
                        nc.vector.memset(slab[:, :, :, 80:81], 1.0)
                        for qk in range(2):
                            c2 = hd + 8 * qk
                            # contiguous i-major load, then XBAR transpose
                            # into [pos128, i] chunks
                            ymqk = ymixp.tile([CT, POS], bf16, tag="ymix")
                            src = y2d[8 * c2:8 * c2 + 8, :, b, :, :]
                            nc.gpsimd.dma_start(
                                ymqk[:],
                                src.rearrange("c1 t h w -> (c1 t) (h w)"))
                            for chk in range(NCH):
                                nc.sync.dma_start_transpose(
                                    slab[:, qk, chk, 0:80],
                                    ymqk[:, 128 * chk:128 * (chk + 1)])
                        ps = ps_b.tile([CT, 2, 162], f32, tag="sps")
                        for qk in range(2):
                            for chk in range(NCH):
                                nc.tensor.matmul(
                                    ps[:, qk],
                                    slab[:, qk, chk, 0:80],
                                    slab[:, :, chk, :],
                                    start=(chk == 0), stop=(chk == NCH - 1))
                        cpc = smallp.tile([CT, 2, 162], f32, tag="cpiece")
                        nc.vector.tensor_copy(cpc[:], ps[:])
                        col = (b * HEADS + hd) * 324
                        nc.gpsimd.dma_start(
                            cin[:, col:col + 324],
                            cpc[:].rearrange("c a i -> c (a i)"))

                for b in range(B):
                    for c2v in range(HEADS):
                        ym = ymixp.tile([CT, POS], bf16, tag="ymix")
                        src = y2d[8 * (16 + c2v):8 * (16 + c2v) + 8, :, b, :, :]
                        nc.gpsimd.dma_start(
                            ym[:], src.rearrange("c1 t h w -> (c1 t) (h w)"))
                        vt = vp.tile([CT, POS], bf16, tag="vt")
                        for chk in range(POS // 512):
                            ps = ps_b.tile([CT, 512], f32, tag="mm512")
                            nc.tensor.matmul(
                                ps[:], wlt[:],
                                ym[:, 512 * chk:512 * (chk + 1)],
                                start=True, stop=True)
                            nc.scalar.activation(
                                vt[:, 512 * chk:512 * (chk + 1)],
                                ps[:], ACTF.Identity, bias=blin[:, 0:1],
                                scale=1.0)
                        nc.gpsimd.dma_start(vdram[:, b, c2v, :], vt[:])

                # ===== stage 3: all-reduce C =====
                nc.gpsimd.collective_compute(
                    "AllReduce", OP.add,
                    replica_groups=[list(range(NCORES))],
                    ins=[cin[:]], outs=[cred_d[:]])
                credf = csbp.tile([CT, B, HEADS, 2, 162], f32, tag="credsb")
                nc.gpsimd.dma_start(
                    credf[:].rearrange("c b h a i -> c (b h a i)"), cred_d[:])
                cred = csbp.tile([CT, B, HEADS, 2, 162], bf16, tag="credbf")
                nc.vector.tensor_copy(cred[:], credf[:])

                # ===== stage 4: per-(b,hd) attention math + attn@v =====
                for b in range(B):
                    for hd in range(HEADS):
                        # M1 = Wl @ [C | s] for q-pass and k-pass
                        psm = ps_b.tile([CT, 2, 162], f32, tag="sps")
                        for qk in range(2):
                            nc.tensor.matmul(psm[:, qk], wlt[:],
                                             cred[:, b, hd, qk],
                                             start=True, stop=True)
                        m1 = smallp.tile([CT, 2, 162], f32, tag="m1")
                        nc.vector.tensor_copy(m1[:], psm[:])
                        # m1[:,0,:] = [WlCqq | Wl s_q | WlCqk | Wl s_q]
                        # m1[:,1,:] = [WlCqk^T| Wl s_k | WlCkk | Wl s_k]

                        # squared norms: diag(Wl C Wl^T) + 2 beta (Wl s) + hw b^2
                        nrm = smallp.tile([CT, 2], f32, tag="nrm")
                        tmp = smallp.tile([CT, CT], f32, tag="dtmp")
                        for qk in range(2):
                            blk = m1[:, qk, 81 * qk:81 * qk + 80]
                            nc.vector.tensor_tensor(
                                tmp[:], blk, wlrm[:], OP.mult)
                            nc.vector.tensor_reduce(
                                nrm[:, qk:qk + 1], tmp[:], op=OP.add, axis=AX)
                            wls = m1[:, qk, 80 + 81 * qk:81 + 81 * qk]
                            nc.vector.scalar_tensor_tensor(
                                nrm[:, qk:qk + 1], wls, blin2[:, 0:1],
                                nrm[:, qk:qk + 1], OP.mult, OP.add)
                        nc.vector.scalar_tensor_tensor(
                            nrm[:], b2hw[:, 0:1].broadcast_to([CT, 2]), 1.0,
                            nrm[:], OP.mult, OP.add)

                        rr = smallp.tile([CT, 2], f32, tag="rr")
                        nc.scalar.activation(rr[:], nrm[:], ACTF.Sqrt)
                        nc.vector.tensor_scalar_max(rr[:], rr[:], EPS)
                        nc.vector.reciprocal(rr[:], rr[:])
                        rq = smallp.tile([CT, 1], f32, tag="rq")
                        nc.vector.tensor_tensor(
                            rq[:], rr[:, 0:1], tcol[:, hd:hd + 1], OP.mult)

                        # G = Wl Cqk Wl^T + (Wl s_q) b^T + b (Wl s_k)^T + hw b b^T
                        m1qk = smallp.tile([CT, CT], bf16, tag="m1qk")
                        nc.vector.tensor_copy(m1qk[:], m1[:, 0, 81:161])
                        pst = ps_a.tile([CT, CT], bf16, tag="trps")
                        nc.tensor.transpose(pst[:], m1qk[:], ident[:])
                        m1t = smallp.tile([CT, CT], bf16, tag="m1t")
                        nc.vector.tensor_copy(m1t[:], pst[:])
                        psg = ps_b.tile([CT, CT], f32, tag="sps")
                        nc.tensor.matmul(psg[:], m1t[:], wlt[:],
                                         start=True, stop=True)
                        g = smallp.tile([CT, CT], f32, tag="g")
                        nc.vector.tensor_copy(g[:], psg[:])
                        nc.vector.scalar_tensor_tensor(
                            g[:], beta[:], m1[:, 0, 80:81], g[:],
                            OP.mult, OP.add)
                        # b (Wl s_k)^T : broadcast Wl s_k along free dim
                        # (via DRAM scratch; partition->free needs flat side)
                        import concourse.bass as _b
                        slot = b * HEADS + hd
                        nc.gpsimd.dma_start(scr[slot, 0, :], m1[:, 1, 161:162])
                        wlsk = smallp.tile([CT, CT], f32, tag="wlsk")
                        sap = scr[slot, 0, :]
                        nc.gpsimd.dma_start(
                            wlsk[:],
                            _b.AP(tensor=sap.tensor, offset=sap.offset,
                                  ap=[[0, CT], [1, CT]]))
                        nc.vector.scalar_tensor_tensor(
                            g[:], wlsk[:], blin[:, 0:1], g[:],
                            OP.mult, OP.add)
                        nc.vector.scalar_tensor_tensor(
                            g[:], beta[:], blhw[:, 0:1], g[:],
                            OP.mult, OP.add)

                        # logits = G * rq[part] * rk[free]; softmax over free
                        rkb = smallp.tile([CT, CT], f32, tag="rkb")
                        nc.gpsimd.dma_start(scr[slot, 1, :], rr[:, 1:2])
                        sap2 = scr[slot, 1, :]
                        nc.gpsimd.dma_start(
                            rkb[:],
                            _b.AP(tensor=sap2.tensor, offset=sap2.offset,
                                  ap=[[0, CT], [1, CT]]))
                        lg = smallp.tile([CT, CT], f32, tag="lg")
                        nc.vector.scalar_tensor_tensor(
                            lg[:], g[:], rq[:, 0:1], rkb[:], OP.mult, OP.mult)
                        mx = smallp.tile([CT, 1], f32, tag="mx")
                        nc.vector.tensor_reduce(mx[:], lg[:], op=OP.max,
                                                axis=AX, negate=True)
                        ex = smallp.tile([CT, CT], f32, tag="ex")
                        ssum = smallp.tile([CT, 1], f32, tag="ssum")
                        nc.scalar.activation(ex[:], lg[:], ACTF.Exp,
                                             bias=mx[:, 0:1], scale=1.0,
                                             accum_out=ssum[:, 0:1])
                        nc.vector.reciprocal(ssum[:], ssum[:])
                        at = smallp.tile([CT, CT], bf16, tag="at")
                        nc.vector.tensor_scalar(at[:], ex[:], ssum[:, 0:1],
                                                None, OP.mult)
                        # transpose attn for attn@v
                        psT = ps_a.tile([CT, CT], bf16, tag="trps")
                        nc.tensor.transpose(psT[:], at[:], ident[:])
                        atT = smallp.tile([CT, CT], bf16, tag="atTs")
                        nc.vector.tensor_copy(atT[:], psT[:])

                        ao = aop.tile([CT, POS], bf16, tag="ao")
                        vt = vp.tile([CT, POS], bf16, tag="vt")
                        nc.gpsimd.dma_start(vt[:], vdram[:, b, hd, :])
                        for chk in range(POS // 512):
                            ps = ps_b.tile([CT, 512], f32, tag="mm512")
                            nc.tensor.matmul(
                                ps[:], atT[:],
                                vt[:, 512 * chk:512 * (chk + 1)],
                                start=True, stop=True)
                            if chk % 2:
                                nc.scalar.copy(
                                    ao[:, 512 * chk:512 * (chk + 1)], ps[:])
                            else:
                                nc.vector.tensor_copy(
                                    ao[:, 512 * chk:512 * (chk + 1)], ps[:])
                        nc.gpsimd.dma_start(aod[b, hd], ao[:])

                # ===== stage 5: W_out =====
                for b in range(B):
                    src_b = aod[b].rearrange("hd (cc t) p -> (hd cc) t p",
                                             cc=8)
                    for t_i in range(T):
                        aocm = aocmp.tile([CDIM, POS], bf16, tag="aocm")
                        nc.gpsimd.dma_start(aocm[:], src_b[:, t_i, :])
                        for chk in range(POS // 512):
                            ps = ps_b.tile([CDIM, 512], f32, tag="mm512")
                            nc.tensor.matmul(
                                ps[:], woutt[:],
                                aocm[:, 512 * chk:512 * (chk + 1)],
                                start=True, stop=True)
                            ot = outp.tile([CDIM, 512], f32, tag="ot")
                            if chk % 2:
                                nc.scalar.copy(ot[:], ps[:])
                            else:
                                nc.vector.tensor_copy(ot[:], ps[:])
                            h_i = 512 * chk // W
                            nc.gpsimd.dma_start(
                                out_d[b, :, t_i, h_i:h_i + 4, :],
                                ot[:].rearrange("c (h w) -> c h w", w=W))
                _rs.close()

    nc.compile()
    return nc


_CACHED = {}


def _get_runner(reps=1):
    if reps in _CACHED:
        return _CACHED[reps]
    nc = build_kernel(reps)
    _CACHED[reps] = nc
    return nc


def kernel(**inputs) -> np.ndarray:
    from concourse.bass_utils import run_bass_kernel_spmd
    nc = _get_runner(1)
    in_maps = _host_prep(
        inputs["x"], inputs["W1"], inputs["Wdw"], inputs["W_lin"],
        inputs["b_lin"], inputs["temperature"], inputs["W_out"])
    res = run_bass_kernel_spmd(nc, in_maps, list(range(NCORES)))
    shards = [res.results[c]["out"] for c in range(NCORES)]
    return np.ascontiguousarray(
        np.concatenate(shards, axis=3).astype(np.float32))
